# revision 14
# baseline (speedup 1.0000x reference)
"""Trainium2 Bass kernel for the MoE-routing execution engine (v2).

Model (per sample): CNN stem (1024->128, 128->128, 3x3) -> routed binary cell
-> 5 routed unary cells -> 1x1 classifier conv -> 2x2 maxpool -> fc1 (25088->
1024) -> relu -> fc2 (1024->28).

Sharding: one fused SPMD launch on 8 cores, data-parallel over batch
(4 samples/core, expert routing resolved host-side by gathering per-sample
expert weights). Pooled features are packed [P, c4, j, 49] and AllGathered
per 2-sample group; group 0's gather overlaps group 1's conv work. fc1 is
output-sharded (128 outs/core over all 32 samples); per-core fc2 partials
[32, 28] are summed on the host.

v2 vs v1 (174036 -> 140851 ns in the TimelineSim cost model):
- pooled features are packed into one contiguous [P, 392] tile per group and
  written to the collective bounce buffer with a single DMA;
- the ag_in DMAs issue from the Activation queue (whose cumulative DMA
  semaphore covers only the pooled path), so each AllGather starts as soon
  as its group's pooling lands instead of waiting out unrelated weight DMAs;
- per-sample weights are staged (binary tile + one tile per unary step) so
  group 1's loads start as group 0 releases each stage, and group 1's later
  stages plus the whole w1 prefetch carry tile_wait_until pins that keep
  their transfers behind ag_in(g0) in the DMA-engine FIFO;
- group 0's AllGather then runs entirely under group 1's conv work, and the
  second AllGather is data-bound (~93us) rather than queue-bound;
- conv-phase PE bubbles trimmed: finer first img/stem chunks (first matmul
  ~3.4us), feats/xcur memsets hoisted ahead of the group loop (they were
  stuck behind the collective issue on the Pool queue), residual convs keep
  the whole res+relu path on DVE (no DVE->Act hop), and the gather pulls
  are split SP/Act with the Act half pinned after ag_in(g1);
- fc1 runs transposed (out [128 outs x 32 samples]): the w1 k-tile is the
  stationary operand and each matmul streams only 32 sample columns, so the
  196-tile contraction takes ~10us instead of ~19us (Ldweights is free;
  per-matmul cost is engine columns + a fixed ~40ns). The fc1 bias rides the
  relu activation's per-partition bias and fc2 consumes the [o, s] layout
  directly, eliminating the transpose/eye/ones-matmul epilogue. fc1 runs
  as two 16-sample column streams: the group-0 half depends only on
  group 0's gather pull and hides under the second AllGather, so only the
  group-1 half (~9us) trails the final pull.
"""

import numpy as np
import ml_dtypes

import concourse.bass as bass
import concourse.mybir as mybir
import concourse.tile as tile
from concourse import bacc
from concourse.bass_utils import run_bass_kernel_spmd

BF16 = ml_dtypes.bfloat16
F32 = mybir.dt.float32
BF = mybir.dt.bfloat16

B, L, HCH, NU, NB, NCLS = 32, 8, 128, 8, 4, 28
NCORES = 8
SPC = B // NCORES          # samples per core = 4
NG = SPC // 2              # groups of 2 samples
NSTEP = L - 3              # unary steps = 5
P = 128

# per-sample routed weight tiles (residuals handled on DVE via gate flags):
#   binary: [0]=presummed 1x1, [1..9]=conv2 taps, [10..18]=conv3 taps
#   unary step s: base+[0..8]=conv1 taps, [9..17]=conv2 taps
BI_TILES = 19
UN_TILES = NSTEP * 18
SAMP_TILES = BI_TILES + UN_TILES  # 109
# bias/flag columns: 0..2 bi b1/b2/b3; 3+2s,4+2s un b1/b2; 13=bi res gate,
# 14+s = unary step res gate
NBCOL = 19

_program_cache = {}
TRACE = False
LAST_EXEC_NS = {}

TAPS = [(t // 3 - 1, t % 3 - 1) for t in range(9)]


def _build_fused_program():
    nc = bacc.Bacc(None, num_devices=NCORES)
    img_in = nc.dram_tensor("img_in", [NG, P, 8, 2, 256], BF, kind="ExternalInput")
    stem1_in = nc.dram_tensor("stem1_in", [8, P, 9 * 128], BF, kind="ExternalInput")
    stem2_in = nc.dram_tensor("stem2_in", [P, 9 * 128], BF, kind="ExternalInput")
    clsw_in = nc.dram_tensor("clsw_in", [P, 4 * 128], BF, kind="ExternalInput")
    sampw_bi_in = nc.dram_tensor("sampw_bi_in", [SPC, P, BI_TILES * 128], BF,
                                 kind="ExternalInput")
    sampw_un_in = nc.dram_tensor("sampw_un_in", [SPC, P, UN_TILES * 128], BF,
                                 kind="ExternalInput")
    biass_in = nc.dram_tensor("biass_in", [SPC, P, NBCOL], F32, kind="ExternalInput")
    biash_in = nc.dram_tensor("biash_in", [P, 6], F32, kind="ExternalInput")
    w1_in = nc.dram_tensor("w1_in", [4, P, 49 * 128], BF, kind="ExternalInput")
    b1_in = nc.dram_tensor("b1_in", [P, 1], F32, kind="ExternalInput")
    w2_in = nc.dram_tensor("w2_in", [P, 28], BF, kind="ExternalInput")
    fc_out = nc.dram_tensor("fc2p_out", [32, 28], F32, kind="ExternalOutput")

    with tile.TileContext(nc) as tc:
        with (
            tc.tile_pool(name="wsh", bufs=1) as wsh,
            tc.tile_pool(name="wsamp", bufs=1) as wsamp,
            tc.tile_pool(name="img", bufs=2) as imgp,
            tc.tile_pool(name="acts", bufs=1) as actp,
            tc.tile_pool(name="persist", bufs=2) as perp,
            tc.tile_pool(name="clsout", bufs=4) as clsp,
            tc.tile_pool(name="pool", bufs=4) as poolp,
            tc.tile_pool(name="fc", bufs=1) as fcp,
            tc.tile_pool(name="dram", bufs=1, space="DRAM") as dram,
            tc.tile_pool(name="psum", bufs=7, space="PSUM") as psum,
            tc.tile_pool(name="psfc", bufs=1, space="PSUM") as psfc,
        ):
            # ---- weight / constant loads, ordered by first use:
            # img g0, stem1, bi weights g0, un weights g0, stem2/biases,
            # img g1, bi/un weights g1, cls, fc weights last
            img_ts = []
            for g in range(NG):
                img_t = imgp.tile([P, 8, 2, 256], BF, tag="img", name=f"img{g}")
                img_ts.append(img_t)
            stem1_w = wsh.tile([P, 72 * 128], BF)
            nc.sync.dma_start(img_ts[0][:, 0:1], img_in[0, :, 0:1])
            nc.sync.dma_start(stem1_w[:, 0:3 * 128], stem1_in[0][:, 0:3 * 128])
            nc.sync.dma_start(stem1_w[:, 3 * 128:9 * 128],
                              stem1_in[0][:, 3 * 128:])
            nc.sync.dma_start(img_ts[0][:, 1:3], img_in[0, :, 1:3])
            nc.sync.dma_start(stem1_w[:, 9 * 128:18 * 128], stem1_in[1])
            nc.sync.dma_start(img_ts[0][:, 3:8], img_in[0, :, 3:8])
            for c8 in range(2, 8):
                nc.sync.dma_start(
                    stem1_w[:, c8 * 9 * 128:(c8 + 1) * 9 * 128], stem1_in[c8])
            stem2_w = wsh.tile([P, 9 * 128], BF)
            cls_w = wsh.tile([P, 4 * 128], BF)
            bias_sh = wsh.tile([P, 6], F32)
            nc.sync.dma_start(stem2_w[:], stem2_in[:])
            nc.sync.dma_start(bias_sh[:], biash_in[:])

            # staged per-sample weights: binary tile + one tile per unary
            # step, so group 1's loads start as group 0 releases each stage
            swbs, swus, bss = [], [], []
            for i in range(SPC):
                swbs.append(wsamp.tile([P, BI_TILES * 128], BF,
                                       tag=f"swb{i % 2}", name=f"swb{i}"))
                swus.append([wsamp.tile([P, 18 * 128], BF,
                                        tag=f"swu{i % 2}_{st}",
                                        name=f"swu{i}_{st}")
                             for st in range(NSTEP)])
                bss.append(wsamp.tile([P, NBCOL], F32, tag=f"bs{i}",
                                      name=f"bs{i}"))

            def load_group(g):
                # first-use order: biases+binary first, then unary stages
                for j in range(2):
                    i = g * 2 + j
                    nc.sync.dma_start(bss[i][:], biass_in[i])
                    nc.sync.dma_start(swbs[i][:], sampw_bi_in[i])
                for st in range(NSTEP):
                    for j in range(2):
                        i = g * 2 + j
                        # group 1's later stages are pinned behind ag_in(g0)
                        # in the DMA FIFO; they are not needed until ~60us+
                        late = g == 1 and st >= 1
                        with tc.tile_wait_until(0.052, enable=late):
                            nc.sync.dma_start(
                                swus[i][st][:],
                                sampw_un_in[i][:, st * 18 * 128:(st + 1) * 18 * 128])

            load_group(0)
            nc.sync.dma_start(img_ts[1][:], img_in[1])
            load_group(1)
            nc.sync.dma_start(cls_w[:], clsw_in[:])

            # fc weights (needed last; big w1 prefetches in 14-k-tile chunks
            # so it never head-of-line blocks later small DMAs)
            w1 = fcp.tile([P, 196 * 128], BF)

            def load_w1(c4s):
                for c4 in c4s:
                    for h in range(2):
                        k0, k1 = h * 25, min(49, h * 25 + 25)
                        nc.sync.dma_start(
                            w1[:, (c4 * 49 + k0) * 128:(c4 * 49 + k1) * 128],
                            w1_in[c4][:, k0 * 128:k1 * 128])

            b1 = fcp.tile([P, 1], F32)
            w2 = fcp.tile([P, 28], BF)
            nc.sync.dma_start(b1[:], b1_in[:])
            nc.sync.dma_start(w2[:], w2_in[:])

            # pooled-feature path: packed [P, c4, j, 49] per group
            ag_ins = [dram.tile([P, 4, 2, 49], BF, name=f"agi{g}")
                      for g in range(NG)]
            ag_outs = [dram.tile([NCORES, P, 4, 2, 49], BF, addr_space="Shared",
                                 name=f"ago{g}") for g in range(NG)]
            pooled_sbs = [fcp.tile([P, 4, 2, 49], BF, tag=f"posb{g}",
                                   name=f"posb{g}") for g in range(NG)]
            # gathered features [P, c4, s, q]; sample s = g*16 + core*2 + j
            pooled_all = fcp.tile([P, 4, 32, 49], BF)

            # transient activation ring (borders zeroed once; writes are
            # interior-only afterwards)
            RING = 12
            ring = [actp.tile([P, 2, 16, 16], BF, tag=f"act{r}", name=f"act{r}")
                    for r in range(RING)]
            for t_ in ring:
                nc.gpsimd.memset(t_[:], 0.0)
            ring_i = [0]
            zeros_t = actp.tile([P, 1, 14, 14], F32, tag="zeros", name="zeros")
            nc.gpsimd.memset(zeros_t[:], 0.0)

            def next_act():
                t_ = ring[ring_i[0] % RING]
                ring_i[0] += 1
                return t_

            def relu_bias(out_ap, ps_ap, bias_ap, engine):
                if engine == "act":
                    nc.scalar.activation(out_ap, ps_ap,
                                         mybir.ActivationFunctionType.Relu,
                                         bias=bias_ap, scale=1.0)
                else:
                    # (ps + bias) max 0 on DVE
                    nc.vector.scalar_tensor_tensor(
                        out_ap, ps_ap, bias_ap, zeros_t[:],
                        mybir.AluOpType.add, mybir.AluOpType.max)

            def conv3x3(dst, dst_j, src, src_j, w_tile, w_off, bias_ap,
                        res_src=None, res_j=None, res_gate=None, nsamp=1,
                        engine="act"):
                """3x3 'SAME' conv (+ gated residual) + bias + relu."""
                ps = psum.tile([P, nsamp, 14, 14], F32, tag="ps", name="ps")
                for t, (dy, dx) in enumerate(TAPS):
                    if src_j is None:
                        rhs = src[:, :, 1 + dy:15 + dy, 1 + dx:15 + dx]
                    else:
                        rhs = src[:, src_j:src_j + 1, 1 + dy:15 + dy, 1 + dx:15 + dx]
                    nc.tensor.matmul(
                        ps[:], w_tile[:, (w_off + t) * 128:(w_off + t + 1) * 128],
                        rhs, start=(t == 0), stop=(t == 8))
                if res_src is not None:
                    # ps += res * gate   (gate is 1.0 / 0.0 per-partition col)
                    nc.vector.scalar_tensor_tensor(
                        ps[:], res_src[:, res_j:res_j + 1, 1:15, 1:15], res_gate,
                        ps[:], mybir.AluOpType.mult, mybir.AluOpType.add)
                if dst_j is None:
                    out_ap = dst[:, :, 1:15, 1:15]
                else:
                    out_ap = dst[:, dst_j:dst_j + 1, 1:15, 1:15]
                relu_bias(out_ap, ps[:], bias_ap, engine)

            feats_ts, xcur_ts = [], []
            for g in range(NG):
                feats_ts.append(perp.tile([P, 2, 16, 16], BF, tag="feats",
                                          name=f"feats{g}"))
                xcur_ts.append(perp.tile([P, 2, 16, 16], BF, tag="xcur",
                                         name=f"xcur{g}"))
                nc.gpsimd.memset(feats_ts[g][:], 0.0)
                nc.gpsimd.memset(xcur_ts[g][:], 0.0)

            for g in range(NG):
                img_t = img_ts[g]
                img_v = img_t[:].rearrange("p c j (h w) -> p c j h w", h=16)

                feats, xcur = feats_ts[g], xcur_ts[g]

                ps = psum.tile([P, 2, 14, 14], F32, tag="ps", name="ps_stem")
                n = 0
                for c8 in range(8):
                    for t, (dy, dx) in enumerate(TAPS):
                        nc.tensor.matmul(
                            ps[:],
                            stem1_w[:, (c8 * 9 + t) * 128:(c8 * 9 + t + 1) * 128],
                            img_v[:, c8, :, 1 + dy:15 + dy, 1 + dx:15 + dx],
                            start=(n == 0), stop=(n == 71))
                        n += 1
                feats_mid = next_act()
                nc.scalar.activation(feats_mid[:, :, 1:15, 1:15], ps[:],
                                     mybir.ActivationFunctionType.Relu,
                                     bias=bias_sh[:, 0:1], scale=1.0)
                conv3x3(feats, None, feats_mid, None, stem2_w, 0,
                        bias_sh[:, 1:2], nsamp=2)

                # two per-sample routed chains, stage-interleaved for PE ILP
                y1s, zs, bxs, srcs = [None, None], [None, None], [None, None], [None, None]
                for j in range(2):
                    i = g * 2 + j
                    sw, bs = swbs[i], bss[i]
                    y1 = next_act()
                    ps1 = psum.tile([P, 1, 14, 14], F32, tag="ps", name="ps_b1")
                    nc.tensor.matmul(ps1[:], sw[:, 0:128],
                                     feats[:, j:j + 1, 1:15, 1:15],
                                     start=True, stop=True)
                    relu_bias(y1[:, j:j + 1, 1:15, 1:15], ps1[:], bs[:, 0:1],
                              "act" if j == 0 else "dve")
                    y1s[j] = y1
                for j in range(2):
                    sw, bs = swbs[g * 2 + j], bss[g * 2 + j]
                    z = next_act()
                    conv3x3(z, j, y1s[j], j, sw, 1, bs[:, 1:2],
                            engine="act" if j == 0 else "dve")
                    zs[j] = z
                for j in range(2):
                    sw, bs = swbs[g * 2 + j], bss[g * 2 + j]
                    bx = next_act()
                    conv3x3(bx, j, zs[j], j, sw, 10, bs[:, 2:3],
                            res_src=y1s[j], res_j=j, res_gate=bs[:, 13:14],
                            engine="dve")
                    srcs[j] = bx
                for s in range(NSTEP):
                    hhs = [None, None]
                    for j in range(2):
                        sw, bs = swus[g * 2 + j][s], bss[g * 2 + j]
                        hh = next_act()
                        conv3x3(hh, j, srcs[j], j, sw, 0,
                                bs[:, 3 + 2 * s:4 + 2 * s],
                                engine="act" if j == 0 else "dve")
                        hhs[j] = hh
                    for j in range(2):
                        sw, bs = swus[g * 2 + j][s], bss[g * 2 + j]
                        xn = xcur if s == NSTEP - 1 else next_act()
                        conv3x3(xn, j, hhs[j], j, sw, 9,
                                bs[:, 4 + 2 * s:5 + 2 * s],
                                res_src=srcs[j], res_j=j, res_gate=bs[:, 14 + s:15 + s],
                                engine="dve")
                        srcs[j] = xn

                pooled_sb = pooled_sbs[g]
                for c4 in range(4):
                    psc = psum.tile([P, 2, 14, 14], F32, tag="ps", name="ps_cls")
                    nc.tensor.matmul(psc[:], cls_w[:, c4 * 128:(c4 + 1) * 128],
                                     xcur[:, :, 1:15, 1:15], start=True, stop=True)
                    co = clsp.tile([P, 2, 14, 14], F32, tag="co", name="co")
                    nc.scalar.activation(co[:], psc[:],
                                         mybir.ActivationFunctionType.Relu,
                                         bias=bias_sh[:, 2 + c4:3 + c4], scale=1.0)
                    m0 = poolp.tile([P, 2, 7, 7], F32, tag="m0", name="m0")
                    m1 = poolp.tile([P, 2, 7, 7], F32, tag="m1", name="m1")
                    nc.vector.scalar_tensor_tensor(
                        m0[:], co[:, :, 0:14:2, 0:14:2], 1.0, co[:, :, 0:14:2, 1:14:2],
                        mybir.AluOpType.mult, mybir.AluOpType.max)
                    nc.vector.scalar_tensor_tensor(
                        m1[:], co[:, :, 1:14:2, 0:14:2], 1.0, co[:, :, 1:14:2, 1:14:2],
                        mybir.AluOpType.mult, mybir.AluOpType.max)
                    nc.vector.scalar_tensor_tensor(
                        pooled_sb[:, c4].rearrange("p j (h w) -> p j h w", h=7),
                        m0[:], 1.0, m1[:],
                        mybir.AluOpType.mult, mybir.AluOpType.max)

                # packed pooled DMA on the DVE queue: DVE issues no other
                # DMAs, so the cumulative DMA semaphore the collective waits
                # on covers exactly the pooled path
                with tc.tile_wait_until(0.045 if g == 0 else 0.088):
                    nc.scalar.dma_start(ag_ins[g][:], pooled_sb[:])
                nc.gpsimd.collective_compute(
                    "AllGather", mybir.AluOpType.bypass,
                    replica_groups=[list(range(NCORES))],
                    ins=[ag_ins[g][:].opt()], outs=[ag_outs[g][:].opt()])
                if g == 0:
                    # w1 loads run while group 1 computes; pinned past the
                    # g0 collective issue so neither the scheduler nor the
                    # cumulative SP DMA semaphore puts them in front of it
                    with tc.tile_wait_until(0.06):
                        load_w1([0, 1, 2, 3])

            # pull gathered features into fc1 sample order (s=g*16+r*2+j);
            # odd cores pull on the Act queue, pinned after ag_in(g1) so the
            # collective's cumulative Act-DMA semaphore never includes them
            for g in range(NG):
                for r in range(NCORES):
                    s0 = g * 16 + r * 2
                    if r % 2 == 0:
                        nc.sync.dma_start(
                            pooled_all[:, :, s0:s0 + 2, :], ag_outs[g][r])
                    else:
                        with tc.tile_wait_until(0.089):
                            nc.scalar.dma_start(
                                pooled_all[:, :, s0:s0 + 2, :], ag_outs[g][r])

            # fc1: out rows r = g*16 + core*2 + j (host unpermutes)
            res = fcp.tile([32, 28], F32)
            # fc1 transposed: out [128 outs(part), 32 samples]; w1 tiles are
            # the stationary operand so each matmul streams only 32 columns
            ps1 = psfc.tile([128, 32], F32, tag="fc", name="fc1ps")
            # two column streams: the g0 half depends only on pull(g0) and
            # runs hidden under AG#1; only the g1 half trails the last pull
            for c0, c1 in ((0, 16), (16, 32)):
                k = 0
                for c4 in range(4):
                    for q in range(49):
                        nc.tensor.matmul(
                            ps1[:, c0:c1], w1[:, k * 128:(k + 1) * 128],
                            pooled_all[:, c4, c0:c1, q],
                            start=(k == 0), stop=(k == 195))
                        k += 1
            relu_s = fcp.tile([P, 32], BF)
            nc.scalar.activation(relu_s[:], ps1[:],
                                 mybir.ActivationFunctionType.Relu,
                                 bias=b1[:, 0:1], scale=1.0)
            ps3 = psfc.tile([32, 28], F32, tag="fc", name="fc3ps")
            nc.tensor.matmul(ps3[:], relu_s[:], w2[:], start=True, stop=True)
            nc.scalar.copy(res[:], ps3[:])
            nc.sync.dma_start(fc_out[:], res[:])
    nc.compile()
    return nc


def _conv_w_tiles(w):
    """[co, ci, 3, 3] -> [ci, 9, co] tap-major lhsT tiles (f32)."""
    return np.ascontiguousarray(w.transpose(1, 2, 3, 0).reshape(
        w.shape[1], 9, w.shape[0]))


def kernel(pInds, img, cnn_w1, cnn_b1, cnn_w2, cnn_b2,
           un_w1, un_b1, un_w2, un_b2,
           bi_w1, bi_b1, bi_w2, bi_b2, bi_w3, bi_b3,
           cls_w1, cls_b1, fc1_w, fc1_b, fc2_w, fc2_b):
    pInds = np.asarray(pInds)
    to_np = lambda a: np.asarray(a, dtype=np.float32)
    img = to_np(img)
    cnn_w1, cnn_b1, cnn_w2, cnn_b2 = map(to_np, (cnn_w1, cnn_b1, cnn_w2, cnn_b2))
    un_w1, un_b1, un_w2, un_b2 = map(to_np, (un_w1, un_b1, un_w2, un_b2))
    bi_w1, bi_b1, bi_w2, bi_b2, bi_w3, bi_b3 = map(
        to_np, (bi_w1, bi_b1, bi_w2, bi_b2, bi_w3, bi_b3))
    cls_w1, cls_b1 = to_np(cls_w1), to_np(cls_b1)
    fc1_w, fc1_b, fc2_w, fc2_b = map(to_np, (fc1_w, fc1_b, fc2_w, fc2_b))

    # ---- shared conv-phase inputs ----
    s1 = cnn_w1.transpose(1, 2, 3, 0).reshape(8, 128, 9, 128)
    stem1_np = np.ascontiguousarray(s1.reshape(8, 128, 9 * 128)).astype(BF16)
    stem2_np = np.ascontiguousarray(
        _conv_w_tiles(cnn_w2).reshape(128, 9 * 128)).astype(BF16)
    clsw_np = np.ascontiguousarray(cls_w1[:, :, 0, 0].T).astype(BF16)
    biash_np = np.zeros((128, 6), np.float32)
    biash_np[:, 0] = cnn_b1
    biash_np[:, 1] = cnn_b2
    biash_np[:, 2:6] = cls_b1.reshape(4, 128).T

    bi_w1s = bi_w1[:, :, :, 0, 0]
    bi_w1p = (bi_w1s[:, :, 0:128] + bi_w1s[:, :, 128:256]).transpose(0, 2, 1)
    bi_w2t = np.stack([_conv_w_tiles(bi_w2[e]) for e in range(NB)])
    bi_w3t = np.stack([_conv_w_tiles(bi_w3[e]) for e in range(NB)])
    un_w1t = np.stack([_conv_w_tiles(un_w1[e]) for e in range(NU)])
    un_w2t = np.stack([_conv_w_tiles(un_w2[e]) for e in range(NU)])

    bidx = pInds[:, 2] - 2 - NU
    uidx = pInds[:, 3:] - 2

    img_pad = np.zeros((B, 1024, 16, 16), dtype=BF16)
    img_pad[:, :, 1:15, 1:15] = img.astype(BF16)

    # fc1 weights, contraction order k = c4*49 + q, p = channel % 128
    w1r = fc1_w.reshape(1024, 4, 128, 49)              # [o, c4, p, q]
    in_maps = []
    for core in range(NCORES):
        sampw = np.zeros((SPC, 128, SAMP_TILES, 128), np.float32)
        biass = np.zeros((SPC, 128, NBCOL), np.float32)
        imgc = np.empty((NG, 2, 8, 128, 256), dtype=BF16)
        for i in range(SPC):
            s = core * SPC + i
            g, j = i // 2, i % 2
            imgc[g, j] = img_pad[s].reshape(8, 128, 256)
            e = int(bidx[s])
            if 0 <= e < NB:
                sampw[i, :, 0] = bi_w1p[e]
                sampw[i, :, 1:10] = bi_w2t[e]
                sampw[i, :, 10:19] = bi_w3t[e]
                biass[i, :, 0] = bi_b1[e]
                biass[i, :, 1] = bi_b2[e]
                biass[i, :, 2] = bi_b3[e]
                biass[i, :, 13] = 1.0
            for st in range(NSTEP):
                u = int(uidx[s, st])
                base = BI_TILES + st * 18
                if 0 <= u < NU:
                    sampw[i, :, base:base + 9] = un_w1t[u]
                    sampw[i, :, base + 9:base + 18] = un_w2t[u]
                    biass[i, :, 3 + 2 * st] = un_b1[u]
                    biass[i, :, 4 + 2 * st] = un_b2[u]
                    biass[i, :, 14 + st] = 1.0
        imgc = np.ascontiguousarray(imgc.transpose(0, 3, 2, 1, 4))  # [NG,P,8,2,256]
        osl = slice(core * 128, (core + 1) * 128)
        w1c = w1r[osl].transpose(2, 1, 3, 0)           # [p, c4, q, o]
        w1c = np.ascontiguousarray(
            w1c.transpose(1, 0, 2, 3).reshape(4, 128, 49 * 128)).astype(BF16)
        in_maps.append({
            "img_in": imgc,
            "stem1_in": stem1_np,
            "stem2_in": stem2_np,
            "clsw_in": clsw_np,
            "sampw_bi_in": np.ascontiguousarray(
                sampw[:, :, :BI_TILES]).reshape(
                    SPC, 128, BI_TILES * 128).astype(BF16),
            "sampw_un_in": np.ascontiguousarray(
                sampw[:, :, BI_TILES:]).reshape(
                    SPC, 128, UN_TILES * 128).astype(BF16),
            "biass_in": biass,
            "biash_in": biash_np,
            "w1_in": w1c,
            "b1_in": fc1_b[osl].reshape(128, 1).astype(np.float32),
            "w2_in": np.ascontiguousarray(fc2_w[:, osl].T).astype(BF16),
        })

    if "fused" not in _program_cache:
        _program_cache["fused"] = _build_fused_program()
    res = run_bass_kernel_spmd(_program_cache["fused"], in_maps,
                               list(range(NCORES)), trace=TRACE)
    if TRACE:
        LAST_EXEC_NS["fused"] = res.exec_time_ns

    acc = np.zeros((32, 28), np.float32)
    for core in range(NCORES):
        acc += res.results[core]["fc2p_out"]
    # device row g*16 + core*2 + j  ->  global sample core*4 + g*2 + j
    out = np.zeros((32, 28), np.float32)
    for g in range(NG):
        for core in range(NCORES):
            for j in range(2):
                out[core * SPC + g * 2 + j] = acc[g * 16 + core * 2 + j]
    out += fc2_b[None, :]
    return out
